# revision 47
# baseline (speedup 1.0000x reference)
"""Trainium2 Bass kernel for nn_Net_84782654423525 (GNN message passing + LSTM).

Strategy (8 NeuronCores, dst-sharded nodes):
  Launch A (mpnn1): per core per timestep, gather X[src] for edges whose dst
    it owns via HBM-source transposed dma_gather (X stored flat [ntok, F]
    fp16 in DRAM).  The timestep's ~85k padded slots are fetched in 10 calls
    (SWDGE descriptor ring caps one gather at ~16k tokens).  Degree-sorted
    node groups keep slot padding at ~6%.  Per-group segment-sum via fp16
    pairwise fold tree on DVE, then relu * (1/cnt) and the folded BN affine;
    h1 shard stored fp16.
  Host: reassemble full h1 (unpermute degree-sorted columns) into a flat
    fp16 token table for launch B.
  Launch B (mpnn2 + 2-layer LSTM + dense): per core, same gather/reduce on
    the h1 table -> h2 (relu+BN), unpermuted on-device via PE-transpose +
    SBUF-gather; LSTM over [h1;h2] with fp16 matmuls (F-on-partition layout
    is matmul-ready), fp32 PSUM, gate activations merged in 1024-wide pairs,
    layer-2 steps interleaved one node-tile behind layer 1; final dense +
    ReLU.

fp16 for gathers/matmuls/elementwise: keeps DVE 2x throughput and
end-to-end rel err at ~5e-3.  f32 where cheap (PSUM accumulation, BN
scale/bias application).
"""

import os
import sys
from contextlib import ExitStack

import numpy as np

sys.path.insert(0, "/opt/trn_rl_repo")

import concourse.bacc as bacc
import concourse.tile as tile
from concourse import mybir
from concourse.bass_utils import run_bass_kernel_spmd

HDT = mybir.dt.float16
F32 = mybir.dt.float32
I16 = mybir.dt.int16
AF = mybir.ActivationFunctionType
EPS = 1e-3
NCORES = 8
NCALL = 10
MAX_GATHER = 16000  # SWDGE ring: one gather must stay under ~16352 tokens

PROFILE = bool(int(os.environ.get("KERNEL_PROFILE", "0")))
LAST_STATS = {}

try:  # trace=True requires antenv.axon_hooks; fall back gracefully
    from antenv.axon_hooks import get_axon_ntff_profile_hook  # noqa: F401
except Exception:
    PROFILE = False


# ---------------------------------------------------------------- host prep

def _pack_idx_blocks(stream, ks):
    """Per-group idx blocks packed for dma_gather: idx i of a block lives at
    [i % 16, i // 16]; blocks concatenated along cols; tiled to 128 rows."""
    blocks = []
    off = 0
    for k in ks:
        n = 128 * int(k)
        s = stream[off : off + n]
        blocks.append(s.reshape(n // 16, 16).T)
        off += n
    m = np.concatenate(blocks, axis=1)  # [16, L/16]
    return np.ascontiguousarray(np.tile(m, (8, 1))).astype(np.int16)


def _plan_t(src, dst, n, ncores, shp, pad_tok, cap=44):
    """Edge plan for one timestep with degree-sorted node groups (tight K).

    Nodes of each core's shard are permuted into degree-descending order so
    that per-group max degree (the padded slot count K) is near the group's
    degree quantile across all cores. Returns (K[NG], streams per core,
    alpha per core (permuted order), perm per core)."""
    sh = n // ncores
    ng = shp // 128
    per_core = []
    for c in range(ncores):
        m = (dst >= c * sh) & (dst < (c + 1) * sh)
        dl = (dst[m] - c * sh).astype(np.int64)
        sl = src[m].astype(np.int64)
        order = np.argsort(dl, kind="stable")
        dl = dl[order]
        sl = sl[order]
        cnt = np.bincount(dl, minlength=sh)
        perm = np.argsort(-cnt, kind="stable")  # natural ids, deg-desc order
        pos_of = np.empty(sh, np.int64)
        pos_of[perm] = np.arange(sh)
        per_core.append((dl, sl, cnt, perm, pos_of))
    npad = shp - sh
    K = np.full(ng, 1, np.int64)
    for dl, sl, cnt, perm, pos_of in per_core:
        cp = np.zeros(shp, np.int64)
        cp[:sh] = cnt[perm]
        # cap the top positions; their overflow edges spill to pad columns
        # (combined on device by one DVE add before the relu)
        cp[sh:] = np.maximum(cp[:npad] - cap, 0)
        cp[:npad] = np.minimum(cp[:npad], cap)
        K = np.maximum(K, cp.reshape(ng, 128).max(1))
    base = np.concatenate([[0], np.cumsum(128 * K)])
    L = int(base[-1])
    streams, alphas, perms = [], [], []
    for dl, sl, cnt, perm, pos_of in per_core:
        stream = np.full(L, pad_tok, np.int64)
        starts = np.concatenate([[0], np.cumsum(cnt)])
        j = np.arange(dl.size) - starts[dl]
        p = pos_of[dl]  # permuted position of each edge's dst
        over = (p < npad) & (j >= cap)
        p = np.where(over, sh + p, p)
        j = np.where(over, j - cap, j)
        pos = base[p // 128] + j * 128 + (p % 128)
        stream[pos] = sl
        streams.append(stream)
        a = np.zeros(shp, np.float32)
        a[:sh] = 1.0 / np.maximum(cnt[perm], 1.0)
        alphas.append(a)
        perms.append(perm)
    return K, streams, alphas, perms


def _split_calls(K, ncall, cap=MAX_GATHER):
    """Partition groups [0, ng) into <= ncall contiguous chunks minimizing
    the max chunk weight (weights 128*K), each chunk <= cap slots."""
    w = (128 * np.asarray(K)).astype(np.int64)
    ng = len(w)
    ncall = min(ncall, ng)

    def n_chunks(capacity):
        chunks, acc = 1, 0
        for wi in w:
            if wi > capacity:
                return None
            if acc + wi > capacity:
                chunks += 1
                acc = int(wi)
            else:
                acc += int(wi)
        return chunks

    lo, hi = int(w.max()), int(w.sum())
    while lo < hi:
        mid = (lo + hi) // 2
        c = n_chunks(mid)
        if c is not None and c <= ncall:
            hi = mid
        else:
            lo = mid + 1
    best = lo
    assert best <= cap, f"cannot split under gather cap: {best} > {cap}"
    bounds = [0]
    acc = 0
    for i, wi in enumerate(w):
        if acc + wi > best:
            bounds.append(i)
            acc = int(wi)
        else:
            acc += int(wi)
    bounds.append(ng)
    return [(bounds[i], bounds[i + 1]) for i in range(len(bounds) - 1)]


# ---------------------------------------------------------- device builders

def _tree_strip(nc, gt, goff, k, out_sl):
    """Fold k slabs of [128,128] fp16 at gt cols [goff, goff+128k) in place;
    final fold writes fp16 into out_sl [128,128]."""

    def slab(a, b):
        return gt[:, 0, goff + a * 128 : goff + b * 128]

    cur = k
    while cur > 2:
        if cur % 2:
            nc.vector.tensor_add(slab(cur - 2, cur - 1), slab(cur - 2, cur - 1),
                                 slab(cur - 1, cur))
            cur -= 1
        h = cur // 2
        nc.vector.tensor_add(slab(0, h), slab(0, h), slab(h, cur))
        cur = h
    if cur == 2:
        nc.vector.tensor_add(out_sl, slab(0, 1), slab(1, 2))
    else:
        nc.vector.tensor_copy(out_sl, slab(0, 1))


def _emit_mpnn(nc, pools, src_ap, idx_d, idx_off, Ks, calls, f, strip,
               steps=()):
    """HBM-source gather + per-group fold for one timestep into strip
    [128, shp] fp16.  After each call, emits the next closure from `steps`
    (previous timestep's LSTM node-tile steps) so the engine queues
    alternate between fold and LSTM work instead of convoying.  Returns new
    idx_off."""
    si = 0
    for g0, g1 in calls:
        Lc = int(128 * Ks[g0:g1].sum())
        if Lc == 0:
            continue
        idxt = pools["idx"].tile([128, Lc // 16], I16, tag="idx")
        nc.sync.dma_start(
            idxt[:], idx_d.ap()[:, idx_off // 16 : (idx_off + Lc) // 16])
        gt = pools["g"].tile([128, 1, Lc], HDT, tag="g")
        nc.gpsimd.dma_gather(gt[:], src_ap, idxt[:], Lc, Lc, f,
                             transpose=True, single_packet=False)
        off = 0
        for g in range(g0, g1):
            kg = int(Ks[g])
            _tree_strip(nc, gt, off, kg, strip[:, g * 128 : (g + 1) * 128])
            off += 128 * kg
        idx_off += Lc
        if si < len(steps):
            steps[si]()
            si += 1
    while si < len(steps):
        steps[si]()
        si += 1
    return idx_off


def _build_launch_a(Ks_all, calls_all, w, f, ntok, shp):
    nc = bacc.Bacc("TRN2", target_bir_lowering=False, debug=False,
                   num_devices=NCORES)
    Ltot = int(sum(128 * K.sum() for K in Ks_all))
    xf_d = nc.dram_tensor("xf", [w, ntok, f], HDT, kind="ExternalInput")
    idx_d = nc.dram_tensor("idx", [128, Ltot // 16], I16, kind="ExternalInput")
    alpha_d = nc.dram_tensor("alpha", [w, 128, shp], HDT, kind="ExternalInput")
    rsg_d = nc.dram_tensor("rsg", [w, 128, 1], F32, kind="ExternalInput")
    bet_d = nc.dram_tensor("bet", [w, 128, 1], F32, kind="ExternalInput")
    h1_d = nc.dram_tensor("h1", [w, 128, shp], HDT, kind="ExternalOutput")

    with tile.TileContext(nc) as tc, ExitStack() as ctx, \
            nc.allow_low_precision(reason="fp16 fold tree by design"):
        pools = {
            "idx": ctx.enter_context(tc.tile_pool(name="idx", bufs=3)),
            "g": ctx.enter_context(tc.tile_pool(name="g", bufs=4)),
            "strip": ctx.enter_context(tc.tile_pool(name="strip", bufs=2)),
            "misc": ctx.enter_context(tc.tile_pool(name="misc", bufs=2)),
        }
        idx_off = 0
        for t in range(w):
            strip = pools["strip"].tile([128, shp], HDT, tag="strip")
            idx_off = _emit_mpnn(nc, pools, xf_d.ap()[t], idx_d, idx_off,
                                 Ks_all[t], calls_all[t], f, strip)
            at = pools["misc"].tile([128, shp], HDT, tag="alpha")
            nc.sync.dma_start(at[:], alpha_d.ap()[t])
            rsgt = pools["misc"].tile([128, 1], F32, tag="rsg")
            nc.sync.dma_start(rsgt[:], rsg_d.ap()[t])
            bett = pools["misc"].tile([128, 1], F32, tag="bet")
            nc.sync.dma_start(bett[:], bet_d.ap()[t])
            with tc.high_priority():
                nc.vector.tensor_add(strip[:, 0:60], strip[:, 0:60],
                                     strip[:, shp - 60 : shp])
                nc.vector.tensor_scalar_max(strip[:], strip[:], 0.0)
                nc.vector.tensor_mul(strip[:], strip[:], at[:])
                h1o = pools["misc"].tile([128, shp], HDT, tag="h1o")
                nc.scalar.activation(h1o[:], strip[:], AF.Identity,
                                     bias=bett[:], scale=rsgt[:])
                nc.sync.dma_start(h1_d.ap()[t], h1o[:])
    nc.compile()
    return nc


def _lstm_step_nt(nc, pools, xa, xb, ka, kb, ra, rb_, h, c, first, ct, nt):
    """One LSTM node-tile step, gate-pair-merged (biases all-zero; asserted
    on host).  xa/xb: fn(nt) -> AP [128, ct] input halves.  h/c: [128,
    ntile, 2*ct] fp16 tiles updated in place."""
    pairs = []
    for pair in range(4):  # keras gate pairs: i, f, g(cell), o
        ps = pools["psum2"].tile([128, 2 * ct], F32, tag="ps")
        for half in range(2):
            gs = slice((2 * pair + half) * 128, (2 * pair + half + 1) * 128)
            o_ap = ps[:, half * ct : (half + 1) * ct]
            if first:
                nc.tensor.matmul(o_ap, ka[:, gs], xa(nt), start=True,
                                 stop=False)
                nc.tensor.matmul(o_ap, kb[:, gs], xb(nt), start=False,
                                 stop=True)
            else:
                # recurrent contribution first: h(t-1) is ready long before
                # this timestep's x (h2n), so PE can run these while the
                # strip/unpermute chain is still in flight
                nc.tensor.matmul(o_ap, ra[:, gs], h[:, nt, 0:ct],
                                 start=True, stop=False)
                nc.tensor.matmul(o_ap, rb_[:, gs], h[:, nt, ct : 2 * ct],
                                 start=False, stop=False)
                nc.tensor.matmul(o_ap, ka[:, gs], xa(nt), start=False,
                                 stop=False)
                nc.tensor.matmul(o_ap, kb[:, gs], xb(nt), start=False,
                                 stop=True)
        gt_ = pools["gate"].tile([128, 2 * ct], HDT, tag="gate")
        func = AF.Tanh if pair == 2 else AF.Sigmoid
        nc.scalar.activation(gt_[:], ps[:], func)
        pairs.append(gt_)
    i_, f_, g_, o_ = pairs
    csl = c[:, nt, :]
    tmp = pools["tmp"].tile([128, 2 * ct], HDT, tag="tmp")
    nc.vector.tensor_mul(tmp[:], i_[:], g_[:])
    if first:
        nc.vector.tensor_copy(csl, tmp[:])
    else:
        nc.vector.tensor_mul(csl, f_[:], csl)
        nc.vector.tensor_add(csl, csl, tmp[:])
    th = pools["tmp"].tile([128, 2 * ct], HDT, tag="tmp")
    nc.scalar.activation(th[:], csl, AF.Tanh)
    nc.vector.tensor_mul(h[:, nt, :], o_[:], th[:])


def _build_launch_b(Ks_all, calls_all, w, f, ntok, shp, u4):
    nc = bacc.Bacc("TRN2", target_bir_lowering=False, debug=False,
                   num_devices=NCORES)
    Ltot = int(sum(128 * K.sum() for K in Ks_all))
    ct = 512
    ntile = shp // ct
    hf_d = nc.dram_tensor("hf", [w, ntok, f], HDT, kind="ExternalInput")
    idx_d = nc.dram_tensor("idx", [128, Ltot // 16], I16, kind="ExternalInput")
    h1t_d = nc.dram_tensor("h1t", [w, 128, shp], HDT, kind="ExternalInput")
    rsg_d = nc.dram_tensor("rsg2", [w, 128, 1], F32, kind="ExternalInput")
    bet_d = nc.dram_tensor("bet2", [w, 128, 1], F32, kind="ExternalInput")
    k1_d = nc.dram_tensor("k1", [256, u4], HDT, kind="ExternalInput")
    r1_d = nc.dram_tensor("r1", [256, u4], HDT, kind="ExternalInput")
    k2_d = nc.dram_tensor("k2", [256, u4], HDT, kind="ExternalInput")
    r2_d = nc.dram_tensor("r2", [256, u4], HDT, kind="ExternalInput")
    wd_d = nc.dram_tensor("wd", [128, 2], HDT, kind="ExternalInput")
    bd_d = nc.dram_tensor("bd", [1, 1], F32, kind="ExternalInput")
    pidx_d = nc.dram_tensor("pinv", [w, 128, shp // 16], I16,
                            kind="ExternalInput")
    ident_d = nc.dram_tensor("ident", [128, 128], HDT, kind="ExternalInput")
    y_d = nc.dram_tensor("y", [1, shp], F32, kind="ExternalOutput")

    with tile.TileContext(nc) as tc, ExitStack() as ctx, \
            nc.allow_low_precision(reason="fp16 state/fold by design"):
        pools = {
            "idx": ctx.enter_context(tc.tile_pool(name="idx", bufs=3)),
            "g": ctx.enter_context(tc.tile_pool(name="g", bufs=4)),
            "strip": ctx.enter_context(tc.tile_pool(name="strip", bufs=2)),
            "h2tok": ctx.enter_context(tc.tile_pool(name="h2tok", bufs=2)),
            "misc": ctx.enter_context(tc.tile_pool(name="misc", bufs=2)),
            "w": ctx.enter_context(tc.tile_pool(name="w", bufs=1)),
            "state": ctx.enter_context(tc.tile_pool(name="state", bufs=1)),
            "gate": ctx.enter_context(tc.tile_pool(name="gate", bufs=6)),
            "tmp": ctx.enter_context(tc.tile_pool(name="tmp", bufs=3)),
            "yd": ctx.enter_context(tc.tile_pool(name="yd", bufs=2)),
            "h1t": ctx.enter_context(tc.tile_pool(name="h1t", bufs=2)),
            "h2n": ctx.enter_context(tc.tile_pool(name="h2n", bufs=2)),
            "psum2": ctx.enter_context(tc.tile_pool(name="psum2", bufs=3,
                                                    space="PSUM")),
            "psd": ctx.enter_context(tc.tile_pool(name="psd", bufs=1,
                                                  space="PSUM")),
        }
        # persistent weights: loaded during timestep 0's gathers so the
        # first idx DMA isn't queued behind them on SP
        wt = {}

        def _load_weights():
            for nm, d in (("k1", k1_d), ("r1", r1_d), ("k2", k2_d),
                          ("r2", r2_d)):
                for half in range(2):
                    tw = pools["w"].tile([128, u4], HDT, tag=f"{nm}{half}")
                    nc.sync.dma_start(tw[:],
                                      d.ap()[half * 128 : (half + 1) * 128])
                    wt[f"{nm}{half}"] = tw
            wdt = pools["w"].tile([128, 2], HDT, tag="wd")
            nc.sync.dma_start(wdt[:], wd_d.ap()[:])
            wt["wd"] = wdt
            bdt = pools["w"].tile([1, 1], F32, tag="bd")
            nc.sync.dma_start(bdt[:], bd_d.ap()[:])
            wt["bd"] = bdt
            identt = pools["w"].tile([128, 128], HDT, tag="ident")
            nc.sync.dma_start(identt[:], ident_d.ap()[:])
            wt["ident"] = identt

        # LSTM state: h and c for both layers, [128, ntile, 2*ct] fp16
        h1s = pools["state"].tile([128, ntile, 2 * ct], HDT, tag="h1s")
        c1s = pools["state"].tile([128, ntile, 2 * ct], HDT, tag="c1s")
        h2s = pools["state"].tile([128, ntile, 2 * ct], HDT, tag="h2s")
        c2s = pools["state"].tile([128, ntile, 2 * ct], HDT, tag="c2s")

        idx_off = 0
        prev_steps = []
        for t in range(w):
            strip = pools["strip"].tile([128, shp], HDT, tag="strip")
            idx_off = _emit_mpnn(nc, pools, hf_d.ap()[t], idx_d, idx_off,
                                 Ks_all[t], calls_all[t], f, strip,
                                 steps=prev_steps)
            if t == 0:
                _load_weights()
            rsgt = pools["misc"].tile([128, 1], F32, tag="rsg")
            nc.sync.dma_start(rsgt[:], rsg_d.ap()[t])
            bett = pools["misc"].tile([128, 1], F32, tag="bet")
            nc.sync.dma_start(bett[:], bet_d.ap()[t])
            h1b = pools["h1t"].tile([128, shp], HDT, tag="h1t")
            nc.sync.dma_start(h1b[:], h1t_d.ap()[t])
            pit = pools["misc"].tile([128, shp // 16], I16, tag="pid")
            nc.sync.dma_start(pit[:], pidx_d.ap()[t])
            # strip -> h2n chain runs at high priority: it gates the next
            # timestep's unpermute on Pool, so it must jump ahead of queued
            # LSTM work on DVE/Act/PE the moment its data is ready.
            with tc.high_priority():
                nc.vector.tensor_add(strip[:, 0:60], strip[:, 0:60],
                                     strip[:, shp - 60 : shp])
                nc.vector.tensor_scalar_max(strip[:], strip[:], 0.0)
                nc.scalar.activation(strip[:], strip[:], AF.Identity,
                                     bias=bett[:], scale=rsgt[:])

                # unpermute h2 columns to natural node order: PE-transpose
                # to token layout, then SBUF-gather with the inverse perm
                h2tok = pools["h2tok"].tile([128, shp], HDT, tag="h2tok")
                for q in range(shp // 512):
                    pst = pools["psd"].tile([128, 512], HDT, tag="pst")
                    for j in range(4):
                        g = q * 4 + j
                        nc.tensor.transpose(pst[:, j * 128 : (j + 1) * 128],
                                            strip[:, g * 128 : (g + 1) * 128],
                                            identt[:])
                    nc.scalar.activation(h2tok[:, q * 512 : (q + 1) * 512],
                                         pst[:], AF.Identity)
                h2n = pools["h2n"].tile([128, 1, shp], HDT, tag="h2n")
                nc.gpsimd.dma_gather(
                    h2n[:], h2tok[:], pit[:], shp, shp, f, transpose=True,
                    sbuf_tokens_per_rank=128, sbuf_free_dim_per_rank=2 * f,
                    single_packet=False)

            # LSTM steps for this timestep: emitted during the NEXT
            # timestep's gather calls (interleaved), so queue order lets
            # that timestep's folds run ahead of this LSTM's tail.
            def _mk_steps(t0, hb, hn):
                x1a = lambda nt: hb[:, nt * ct : (nt + 1) * ct]
                x1b = lambda nt: hn[:, 0, nt * ct : (nt + 1) * ct]
                x2a = lambda nt: h1s[:, nt, 0:ct]
                x2b = lambda nt: h1s[:, nt, ct : 2 * ct]

                def step(k):
                    if k < ntile:
                        _lstm_step_nt(nc, pools, x1a, x1b, wt["k10"][:],
                                      wt["k11"][:], wt["r10"][:],
                                      wt["r11"][:], h1s, c1s, t0 == 0, ct, k)
                    if k >= 1:
                        _lstm_step_nt(nc, pools, x2a, x2b, wt["k20"][:],
                                      wt["k21"][:], wt["r20"][:],
                                      wt["r21"][:], h2s, c2s, t0 == 0, ct,
                                      k - 1)

                return [(lambda k=k: step(k)) for k in range(ntile + 1)]

            prev_steps = _mk_steps(t, h1b, h2n)
        for s in prev_steps:
            s()

        # dense head: y = relu(hT @ wd + bd)
        for nt in range(ntile):
            ps = pools["psd"].tile([1, ct], F32, tag="psy")
            nc.tensor.matmul(ps[:], wt["wd"][:, 0:1], h2s[:, nt, 0:ct], start=True,
                             stop=False)
            nc.tensor.matmul(ps[:], wt["wd"][:, 1:2], h2s[:, nt, ct : 2 * ct],
                             start=False, stop=True)
            yt = pools["yd"].tile([1, ct], F32, tag="y")
            nc.scalar.activation(yt[:], ps[:], AF.Relu,
                                 bias=wt["bd"][:, 0:1])
            nc.sync.dma_start(y_d.ap()[:, nt * ct : (nt + 1) * ct], yt[:])
    nc.compile()
    return nc


# ----------------------------------------------------------------- kernel()

def kernel(**inputs):
    X = np.asarray(inputs["X"], np.float32)
    edge_src = np.asarray(inputs["edge_src"])
    edge_dst = np.asarray(inputs["edge_dst"])
    w, n, f = X.shape
    u4 = int(np.asarray(inputs["k1"]).shape[1])
    sh = n // NCORES
    ng = max(1, (sh + 127) // 128)
    shp = ng * 128
    ntok = n + 1
    pad_tok = n

    # fold BN params
    rsg1 = (np.asarray(inputs["gamma1"], np.float32)
            / np.sqrt(np.asarray(inputs["var1"], np.float32) + EPS))
    bet1 = (np.asarray(inputs["beta1"], np.float32)
            - np.asarray(inputs["mean1"], np.float32) * rsg1)
    rsg2 = (np.asarray(inputs["gamma2"], np.float32)
            / np.sqrt(np.asarray(inputs["var2"], np.float32) + EPS))
    bet2 = (np.asarray(inputs["beta2"], np.float32)
            - np.asarray(inputs["mean2"], np.float32) * rsg2)

    assert np.all(np.asarray(inputs["b1"]) == 0) and \
        np.all(np.asarray(inputs["b2"]) == 0), "nonzero LSTM bias unsupported"

    # edge plans
    Ks_all, streams_all, alphas_all, perms_all, calls_all = [], [], [], [], []
    for t in range(w):
        K, streams, alphas, perms = _plan_t(np.asarray(edge_src[t]),
                                            np.asarray(edge_dst[t]),
                                            n, NCORES, shp, pad_tok)
        Ks_all.append(K)
        streams_all.append(streams)
        alphas_all.append(alphas)
        perms_all.append(perms)
        calls_all.append(_split_calls(K, NCALL))

    # packed inputs
    xf = np.zeros((w, ntok, f), np.float16)
    xf[:, :n] = X.astype(np.float16)
    idx_packed = []
    alpha_packed = []
    for c in range(NCORES):
        idx_packed.append(np.concatenate(
            [_pack_idx_blocks(streams_all[t][c], Ks_all[t]) for t in range(w)],
            axis=1))
        alpha_packed.append(np.stack(
            [np.tile(alphas_all[t][c].astype(np.float16), (128, 1))
             for t in range(w)]))
    rsg1_in = rsg1.reshape(w, 128, 1)
    bet1_in = bet1.reshape(w, 128, 1)

    # ---- launch A
    nc_a = _build_launch_a(Ks_all, calls_all, w, f, ntok, shp)
    in_maps_a = [
        dict(xf=xf, idx=idx_packed[c], alpha=alpha_packed[c],
             rsg=rsg1_in, bet=bet1_in)
        for c in range(NCORES)
    ]
    LAST_STATS["nc_a"] = nc_a
    res_a = run_bass_kernel_spmd(nc_a, in_maps_a, core_ids=list(range(NCORES)),
                                 trace=PROFILE)
    LAST_STATS["a_exec_ns"] = res_a.exec_time_ns
    h1_shards = [res_a.results[c]["h1"] for c in range(NCORES)]  # [w,128,shp] f16

    # ---- host exchange (unpermute launch-A columns back to natural order)
    h1_full = np.empty((w, n, f), np.float32)
    for c in range(NCORES):
        for t in range(w):
            h1_full[t, c * sh + perms_all[t][c], :] = (
                h1_shards[c][t, :, :sh].T.astype(np.float32))
    hf = np.zeros((w, ntok, f), np.float16)
    hf[:, :n] = h1_full.astype(np.float16)
    h1t = []
    for c in range(NCORES):
        v = np.zeros((w, 128, shp), np.float16)
        v[:, :, :sh] = h1_full[:, c * sh : (c + 1) * sh, :].transpose(0, 2, 1)
        h1t.append(v)
    # inverse-permutation gather indices for launch B's h2 unpermute
    pinv_packed = []
    for c in range(NCORES):
        blocks = []
        for t in range(w):
            pos_of = np.zeros(shp, np.int64)
            pos_of[perms_all[t][c]] = np.arange(sh)
            blocks.append(_pack_idx_blocks(pos_of, [shp // 128]))
        pinv_packed.append(np.stack(blocks))
    ident = np.eye(128, dtype=np.float16)

    # ---- launch B
    k1 = np.asarray(inputs["k1"], np.float32).astype(np.float16)
    r1 = np.asarray(inputs["r1"], np.float32).astype(np.float16)
    k2 = np.asarray(inputs["k2"], np.float32).astype(np.float16)
    r2 = np.asarray(inputs["r2"], np.float32).astype(np.float16)
    wd = np.asarray(inputs["wd"], np.float32).reshape(2, 128).T.copy().astype(
        np.float16)
    bd = np.asarray(inputs["bd"], np.float32).reshape(1, 1)
    rsg2_in = rsg2.reshape(w, 128, 1)
    bet2_in = bet2.reshape(w, 128, 1)

    nc_b = _build_launch_b(Ks_all, calls_all, w, f, ntok, shp, u4)
    in_maps_b = [
        dict(hf=hf, idx=idx_packed[c], h1t=h1t[c], rsg2=rsg2_in,
             bet2=bet2_in, k1=k1, r1=r1, k2=k2, r2=r2, wd=wd,
             bd=bd, pinv=pinv_packed[c], ident=ident)
        for c in range(NCORES)
    ]
    LAST_STATS["nc_b"] = nc_b
    res_b = run_bass_kernel_spmd(nc_b, in_maps_b, core_ids=list(range(NCORES)),
                                 trace=PROFILE)
    LAST_STATS["b_exec_ns"] = res_b.exec_time_ns

    out = np.empty((n, 1), np.float32)
    for c in range(NCORES):
        out[c * sh : (c + 1) * sh, 0] = res_b.results[c]["y"][0, :sh]
    return out
